# revision 6
# baseline (speedup 1.0000x reference)
"""GQA attention (B=1, S=2048, D=4096, HQ=32, HKV=8, HD=128) + RoPE + causal,
tensor-parallel over heads on 8 TRN2 NeuronCores.

v2 layout strategy (vs v1): single c-outer loop fuses the K/V projection
(old phase0) with the Q projection so xT streams exactly once per core
(16 MB instead of 32 MB) and the PE never waits on a separate projection
phase.  Per sq-tile c: project k/v/q (6 PSUM accumulators over 32 k-tiles,
x/wq/wkv batched 8 k-tiles per DMA), rope, causally-skipped attention
(scoresT per 128-sk block, exp on ACT pipelined one block deep, PV + ones
row-sum accumulate, reciprocal normalize), then ONE AllGather per tile
(4 heads batched, [512,512] bf16 per rank) so the collective stream starts
~70us in and stays busy.  wo phases (column-sharded wo resident in SBUF)
are deferred: wo(0) after attention(2), wo(1..3) after attention(3), so
every AG completes before its wo phase needs it and the AG stream is fully
hidden under PE work.  All PE matmuls bf16 (f32 PSUM)."""

import math

import ml_dtypes
import numpy as np

import concourse.bass as bass
import concourse.tile as tile
from concourse import bacc, mybir
from concourse.bass_utils import run_bass_kernel_spmd

F32 = mybir.dt.float32
BF16 = mybir.dt.bfloat16


def dedup_ldweights(nc):
    """Remove InstLdweights whose weights AP identically matches the previous
    InstLdweights on the PE queue with only InstMatmult instructions between:
    the PE array already holds those weights, so the reload's 128 cycles are
    pure overhead.  The dropped load's dependency edges move to the next PE
    instruction so no synchronization is lost."""
    deleted = {}
    for f in nc.m.functions:
        for bb in f.blocks:
            insts = bb.instructions
            out = []
            last_key = None
            last_name = None
            pending_edges = []
            for i in insts:
                tn = type(i).__name__
                eng = getattr(i, "engine", None)
                is_pe = eng is not None and "PE" in str(eng)
                if tn == "InstLdweights":
                    key = str(i.ins[0])
                    if key == last_key:
                        deleted[i.name] = last_name
                        pending_edges.extend(i.dependency_edges())
                        continue
                    last_key = key
                    last_name = i.name
                elif tn == "InstMatmult":
                    if getattr(i, "is_transpose", False):
                        last_key = None
                    for nm, info in pending_edges:
                        i.add_dependency(nm, info)
                    pending_edges = []
                elif is_pe:
                    last_key = None
                out.append(i)
            assert not pending_edges
            bb.instructions = out
    if deleted:
        for f in nc.m.functions:
            for bb in f.blocks:
                for i in bb.instructions:
                    try:
                        i.remap_dependency_names(deleted)
                    except Exception:
                        pass
    return len(deleted)

S = 2048
D = 4096
HQ, HKV, HD = 32, 8, 128
HL = HQ // 8            # 4 local q heads per core
SQT = 512               # sq tile
NSQ = S // SQT          # 4
NK = D // 128           # 32 contraction k-tiles
NSK = S // 128          # 16 sk tiles
N_CORES = 8
AF = mybir.ActivationFunctionType
ALU = mybir.AluOpType


def build_nc():
    nc = bacc.Bacc(num_devices=N_CORES, num_swdge_queues=4)

    # all inputs are host-pre-tiled so every DMA reads >=4KB contiguous per
    # partition (1KB descriptors cap a queue at ~90 GB/s; 4KB+ runs at line
    # rate).  Row blocks of 128 are partition-major: row = block*128 + p,
    # cols = chunk-within-block * orig_cols + col.
    xTt = nc.declare_dram_parameter("xTt", [NSQ * 8 * 128, 4 * SQT], BF16,
                                    isOutput=False)
    wqt = nc.declare_dram_parameter("wqt", [8 * 128, 4 * HL * HD], BF16,
                                    isOutput=False)
    wkvt = nc.declare_dram_parameter("wkvt", [8 * 128, 4 * 2 * HD], BF16,
                                     isOutput=False)
    wot = nc.declare_dram_parameter("wot", [128, NK * SQT], BF16,
                                    isOutput=False)  # col shard
    cos2 = nc.declare_dram_parameter("cos2", [128, S], BF16, isOutput=False)
    sin2 = nc.declare_dram_parameter("sin2", [128, S], BF16, isOutput=False)
    mtd = nc.declare_dram_parameter("mtd", [128, NSK * 128], BF16,
                                    isOutput=False)
    swp = nc.declare_dram_parameter("swp", [128, 128], BF16, isOutput=False)
    idn = nc.declare_dram_parameter("idn", [128, 128], BF16, isOutput=False)
    # transposed: rows = this core's 512 wo columns, cols = full sequence
    out = nc.declare_dram_parameter("out", [SQT, S], BF16, isOutput=True)

    with tile.TileContext(nc) as tc:
        with tc.tile_pool(name="const", bufs=1) as constp, \
             tc.tile_pool(name="wqp", bufs=3) as wqp, \
             tc.tile_pool(name="wkvp", bufs=1) as wkvp, \
             tc.tile_pool(name="wob", bufs=1) as wob, \
             tc.tile_pool(name="xp", bufs=3) as xp, \
             tc.tile_pool(name="qp", bufs=2) as qp, \
             tc.tile_pool(name="ep", bufs=5) as ep, \
             tc.tile_pool(name="eap", bufs=2) as eap, \
             tc.tile_pool(name="tp", bufs=2) as tp, \
             tc.tile_pool(name="agp", bufs=8) as agp, \
             tc.tile_pool(name="op", bufs=2) as opool, \
             tc.tile_pool(name="outp", bufs=2) as outp, \
             tc.tile_pool(name="ps", bufs=8, space="PSUM") as ps, \
             tc.tile_pool(name="dram", bufs=1, space="DRAM") as dramp:

            # ---- weights: 4 k-tiles per DMA.  The gpsimd software-DGE
            # ---- queue sustains ~200 GB/s vs ~60-85 for the sync/scalar
            # ---- hardware-DGE queues, so c=0's x+wq (needed inside 50us)
            # ---- interleave on swdge; wkv rides the scalar queue.
            # ---- wq is re-streamed every tile (ring of 3 chunks) — the
            # ---- 3 MB of SBUF this frees funds the deep AG-fetch ring ----
            def wq_stream():
                ts = [wqp.tile([128, 4 * HL * HD], BF16, tag="wq",
                               name="wqs") for j in range(8)]
                return ts

            def wq_dma(ts, j):
                nc.gpsimd.dma_start(out=ts[j],
                                    in_=wqt[j * 128:(j + 1) * 128, :])

            wkv_t = [wkvp.tile([128, 4 * 2 * HD], BF16, tag=f"wkv{j}",
                               name=f"wkv{j}") for j in range(8)]
            xts0 = [xp.tile([128, 4 * SQT], BF16, tag="xt", name=f"x0{j}")
                    for j in range(8)]
            wq0 = wq_stream()
            for j in range(8):
                # odd x chunks ride the (otherwise idle) sync hw queue so
                # c=0's 8 MB feed isn't serialized on the single swdge queue
                xeng = nc.gpsimd if j % 2 == 0 else nc.sync
                xeng.dma_start(out=xts0[j],
                               in_=xTt[j * 128:(j + 1) * 128, :])
                wq_dma(wq0, j)
                nc.scalar.dma_start(out=wkv_t[j],
                                    in_=wkvt[j * 128:(j + 1) * 128, :])

            # ---- constants (scalar queue, parallel with sync queue) ----
            cos_sb = constp.tile([128, S], BF16)
            nc.scalar.dma_start(out=cos_sb, in_=cos2[:, :])
            sin_sb = constp.tile([128, S], BF16)
            nc.scalar.dma_start(out=sin_sb, in_=sin2[:, :])
            mtd_sb = constp.tile([128, NSK * 128], BF16)
            nc.scalar.dma_start(out=mtd_sb, in_=mtd[:, :])
            swp_sb = constp.tile([128, 128], BF16)
            nc.scalar.dma_start(out=swp_sb, in_=swp[:, :])
            idn_sb = constp.tile([128, 128], BF16)
            nc.scalar.dma_start(out=idn_sb, in_=idn[:, :])
            ones_f = constp.tile([128, 128], F32)
            nc.vector.memset(ones_f, 1.0)
            allones = constp.tile([128, 128], BF16)
            nc.scalar.activation(allones, ones_f, AF.Copy)

            kT_sb = constp.tile([128, S], BF16)      # kv head, feature-major
            v_sb = constp.tile([128, S], BF16)       # [sk%128, (sk//128)*128 + d]

            # wo_sb is loaded during the c=1 section: at c=0 the 4 MB load
            # would compete with the projection feed for HBM bandwidth
            wo_sb = wob.tile([128, NK * SQT], BF16)

            ag_in = [dramp.tile([HL * 128, SQT], BF16, name=f"agin{c}")
                     for c in range(NSQ)]
            ag_out = [dramp.tile([HQ * 128, SQT], BF16, addr_space="Shared",
                                 name=f"agout{c}") for c in range(NSQ)]

            def rope(ps_t, cos_cols, sin_cols, dst):
                """dst[bf16 sbuf 128xSQT] = cos*ps + signed-half-swap(sin*ps).

                t2 (sin*q) is emitted first so the PE swap matmul only waits
                on one DVE op; t1 (cos*q) runs while the PE swaps."""
                t2 = tp.tile([128, SQT], BF16, tag="t2")
                nc.vector.tensor_tensor(out=t2, in0=ps_t, in1=sin_cols,
                                        op=ALU.mult)
                t2s = ps.tile([128, SQT], F32, tag="ps", name="t2s")
                nc.tensor.matmul(t2s, swp_sb, t2, start=True, stop=True)
                t1 = tp.tile([128, SQT], BF16, tag="t1")
                nc.vector.tensor_tensor(out=t1, in0=ps_t, in1=cos_cols,
                                        op=ALU.mult)
                nc.vector.tensor_tensor(out=dst, in0=t1, in1=t2s, op=ALU.add)

            def attention_head(c, h, qT_sb, fin_prev):
                """Emits head (c,h); returns a finalize closure (softmax
                denominator + normalize + ag_in write) that the CALLER must
                emit later — it is passed back in as fin_prev of the next
                head and emitted after 3 of its blocks, so the PE never
                waits on the accumulate chain.

                e blocks are accumulated in f32 split across DVE (even
                blocks) and GpSimd (odd blocks) — each engine then runs at
                half the block rate and keeps up; ONE ones-matmul per head
                broadcasts the partition-sum instead of one per 128-sk
                block."""
                nsk_here = 4 * c + 4
                o_ps = ps.tile([128, SQT], F32, tag="ps", name=f"o{c}{h}")
                e_acc_d = eap.tile([128, SQT], F32, tag="eaccd")
                e_acc_g = eap.tile([128, SQT], F32, tag="eaccg")
                if c == 0:
                    # odd chain starts at kt2=1 which is diagonal (col0=128)
                    nc.gpsimd.memset(e_acc_g, 0.0)

                def flush(pending):
                    pe, pc, pk = pending
                    nc.tensor.matmul(
                        o_ps[:, pc:], v_sb[:, pk * 128:(pk + 1) * 128],
                        pe[:, pc:], start=(pk == 0),
                        stop=(pk == nsk_here - 1))

                # PV flushes lag the score/exp stream TWO blocks so the
                # mask-add + exp chain (~1.2us on diagonal blocks) never
                # gates the PE, which has ~1us of matmuls per 2 blocks
                pending = []  # (e_sb, col0, kt2) awaiting PV
                for kt2 in range(nsk_here):
                    m = kt2 - 4 * c
                    col0 = 128 * m if m > 0 else 0
                    s_ps = ps.tile([128, SQT], F32, tag="ps", name="s_ps")
                    nc.tensor.matmul(
                        s_ps[:, col0:], kT_sb[:, kt2 * 128:(kt2 + 1) * 128],
                        qT_sb[:, h, col0:], start=True, stop=True)
                    if m >= 0:
                        nc.vector.tensor_tensor(
                            out=s_ps[:, col0:col0 + 128],
                            in0=s_ps[:, col0:col0 + 128],
                            in1=mtd_sb[:, kt2 * 128:(kt2 + 1) * 128],
                            op=ALU.add)
                    e_sb = ep.tile([128, SQT], BF16, tag="e")
                    nc.scalar.activation(e_sb[:, col0:], s_ps[:, col0:],
                                         AF.Exp)
                    eng = nc.vector if kt2 % 2 == 0 else nc.gpsimd
                    acc = e_acc_d if kt2 % 2 == 0 else e_acc_g
                    if kt2 == 0 or (kt2 == 1 and c > 0):
                        eng.tensor_copy(out=acc, in_=e_sb)
                    else:
                        eng.tensor_tensor(out=acc[:, col0:],
                                          in0=acc[:, col0:],
                                          in1=e_sb[:, col0:], op=ALU.add)
                    pending.append((e_sb, col0, kt2))
                    if len(pending) > 2:
                        flush(pending.pop(0))
                    if kt2 == 2 and fin_prev is not None:
                        fin_prev()
                for p in pending:
                    flush(p)

                def finalize():
                    ea_bf = eap.tile([128, SQT], BF16, tag="eaccb")
                    nc.vector.tensor_tensor(out=ea_bf, in0=e_acc_d,
                                            in1=e_acc_g, op=ALU.add)
                    sum_ps = ps.tile([128, SQT], F32, tag="ps",
                                     name=f"sb{c}{h}")
                    nc.tensor.matmul(sum_ps, allones, ea_bf,
                                     start=True, stop=True)
                    rec_sb = opool.tile([128, SQT], F32, tag="rcb")
                    nc.vector.reciprocal_approx_fast(out=rec_sb, in_=sum_ps)
                    on_sb = opool.tile([128, SQT], BF16, tag="on")
                    nc.vector.tensor_tensor(out=on_sb, in0=rec_sb, in1=o_ps,
                                            op=ALU.mult)
                    # gpsimd (not sync): the sync engine must stay empty of
                    # mid-kernel work so hoisted wo-phase fetches can block
                    # on AG completion there without starving anything
                    nc.gpsimd.dma_start(
                        out=ag_in[c][h * 128:(h + 1) * 128, :], in_=on_sb)
                return finalize

            def wo_pair(c0, c1):
                """Output projection for sq tiles {c0, c1} with the wo
                stationary tile shared across both (dedup_ldweights drops the
                second load).  Transposed output: ps[oc][ci] = [128 wo-cols,
                512 sq] accumulated over all 32 contraction blocks g — 8 PSUM
                banks live for the whole pair.  AG(c) output streams through
                SBUF in [128, 8, 512] chunks (1 MB, ring) fetched per
                (c, gq)."""
                cs = (c0, c1)
                o_ps = [[ps.tile([128, SQT], F32, tag="ps",
                                 name=f"wops{c}{oc}") for c in cs]
                        for oc in range(4)]
                for gq in range(4):
                    ag_ts = []
                    for ci, c in enumerate(cs):
                        ag_t = agp.tile([128, 8, SQT], BF16, tag="ag",
                                        name=f"agt{c}{gq}")
                        eng = nc.sync if ci == 0 else nc.scalar
                        eng.dma_start(
                            out=ag_t,
                            in_=ag_out[c][gq * 1024:(gq + 1) * 1024, :]
                            .rearrange("(g p) n -> p g n", p=128))
                        ag_ts.append(ag_t)
                    for oc in range(4):
                        for g8 in range(8):
                            g = gq * 8 + g8
                            for ci in range(2):
                                nc.tensor.matmul(
                                    o_ps[oc][ci],
                                    wo_sb[:, g * SQT + oc * 128:
                                          g * SQT + (oc + 1) * 128],
                                    ag_ts[ci][:, g8, :],
                                    start=(g == 0), stop=(g == NK - 1))
                for oc in range(4):
                    for ci, c in enumerate(cs):
                        ob = outp.tile([128, SQT], BF16, tag="ob")
                        nc.scalar.activation(ob, o_ps[oc][ci], AF.Copy)
                        nc.scalar.dma_start(
                            out=out[oc * 128:(oc + 1) * 128,
                                    c * SQT:(c + 1) * SQT],
                            in_=ob)

            for c in range(NSQ):
                s0 = c * SQT
                cse = (slice(None), slice(s0, s0 + SQT))
                if c == 0:
                    xts = xts0
                    wqs = wq0
                else:
                    if c == 1:
                        nc.gpsimd.dma_start(out=wo_sb, in_=wot[:, :])
                    xts = xts_next
                    wqs = wq_next

                k_ps = ps.tile([128, SQT], F32, tag="ps", name=f"kps{c}")
                v_ps = ps.tile([128, SQT], F32, tag="ps", name=f"vps{c}")
                q_ps = [ps.tile([128, SQT], F32, tag="ps", name=f"qps{c}{h}")
                        for h in range(HL)]
                for kt in range(NK):
                    t = kt % 4
                    xt = xts[kt // 4][:, t * SQT:(t + 1) * SQT]
                    nc.tensor.matmul(
                        k_ps, wkv_t[kt // 4][:, t * 2 * HD:t * 2 * HD + HD],
                        xt, start=(kt == 0), stop=(kt == NK - 1))
                    nc.tensor.matmul(
                        v_ps,
                        wkv_t[kt // 4][:, t * 2 * HD + HD:(t + 1) * 2 * HD],
                        xt, start=(kt == 0), stop=(kt == NK - 1))
                    for h in range(HL):
                        nc.tensor.matmul(
                            q_ps[h],
                            wqs[kt // 4][:, t * HL * HD + h * 128:
                                         t * HL * HD + (h + 1) * 128],
                            xt, start=(kt == 0), stop=(kt == NK - 1))

                # drain v_ps first (frees its PSUM slot), rope k into kT_sb,
                # transpose v into v_sb
                # v drain/transpose runs on ACT (idle at boundaries) so the
                # DVE is free for the rope chains.  For c>0, rope(q0) goes
                # FIRST: attention h0's early blocks read only older tiles'
                # kT/v, so only the q0 chain gates the first score matmul —
                # rope(k) and the v transposes finish under h0's early blocks
                vt_sb = tp.tile([128, SQT], BF16, tag="vt", bufs=2)
                nc.scalar.activation(vt_sb, v_ps, AF.Copy)
                qT_sb = qp.tile([128, HL, SQT], BF16, tag="qT")
                if c > 0:
                    rope(q_ps[0], cos_sb[cse], sin_sb[cse], qT_sb[:, 0, :])
                    rope(k_ps, cos_sb[cse], sin_sb[cse], kT_sb[cse])
                else:
                    rope(k_ps, cos_sb[cse], sin_sb[cse], kT_sb[cse])
                    rope(q_ps[0], cos_sb[cse], sin_sb[cse], qT_sb[:, 0, :])
                for sb in range(SQT // 128):
                    vp = ps.tile([128, 128], BF16, tag="ps", name="vtp")
                    nc.tensor.transpose(vp, vt_sb[:, sb * 128:(sb + 1) * 128],
                                        idn_sb)
                    nc.scalar.activation(
                        v_sb[:, (4 * c + sb) * 128:(4 * c + sb + 1) * 128],
                        vp, AF.Copy)
                rope(q_ps[1], cos_sb[cse], sin_sb[cse], qT_sb[:, 1, :])

                # prefetch next tile's x and wq during attention(c) on the
                # fast gpsimd swdge queue (~200 GB/s vs 60-85 for hw queues)
                if c + 1 < NSQ:
                    cn = c + 1
                    xts_next = [xp.tile([128, 4 * SQT], BF16, tag="xt",
                                        name=f"x{cn}{j}") for j in range(8)]
                    wq_next = wq_stream()
                    for j in range(8):
                        nc.gpsimd.dma_start(
                            out=xts_next[j],
                            in_=xTt[(cn * 8 + j) * 128:(cn * 8 + j + 1) * 128,
                                    :])
                        wq_dma(wq_next, j)
                fin = None
                for h in range(HL):
                    fin = attention_head(c, h, qT_sb, fin)
                    if h + 2 < HL:
                        rope(q_ps[h + 2], cos_sb[cse], sin_sb[cse],
                             qT_sb[:, h + 2, :])
                fin()

                nc.gpsimd.collective_compute(
                    "AllGather", ALU.bypass,
                    replica_groups=[list(range(N_CORES))],
                    ins=[ag_in[c].opt()], outs=[ag_out[c].opt()])

            # wo pairs at the tail: every AG trigger is already issued; pair
            # {0,1} needs only AG(0,1) and its ~60us of work hides AG(3)
            wo_pair(0, 1)
            wo_pair(2, 3)

    dedup_ldweights(nc)
    nc.finalize()
    return nc


_CACHE = {}


def _tile_rows(a, chunk=4):
    """[D, C] -> [D//(128*chunk) * 128, chunk*C]: row blocks partition-major
    so each DMA partition reads chunk*C contiguous elements."""
    dd, cc = a.shape
    nj = dd // (128 * chunk)
    return np.ascontiguousarray(
        a.reshape(nj, chunk, 128, cc).transpose(0, 2, 1, 3).reshape(
            nj * 128, chunk * cc))


def _host_prep(x, wq, wk, wv, wo, cos, sin, mask):
    perm = np.concatenate([np.arange(0, HD, 2), np.arange(1, HD, 2)])
    bf = ml_dtypes.bfloat16
    xT = np.ascontiguousarray(x.reshape(S, D).T).astype(bf)
    xTt = np.concatenate(
        [_tile_rows(xT[:, c * SQT:(c + 1) * SQT]) for c in range(NSQ)],
        axis=0)
    cos2 = np.ascontiguousarray(np.vstack([cos.T, cos.T])).astype(bf)
    sin2 = np.ascontiguousarray(np.vstack([sin.T, sin.T])).astype(bf)
    mtd = np.ascontiguousarray(np.concatenate(
        [mask[k * 128:(k + 1) * 128, k * 128:(k + 1) * 128].T
         for k in range(NSK)], axis=1)).astype(bf)
    swp = np.zeros((128, 128), np.float32)
    for mcol in range(64):
        swp[mcol + 64, mcol] = -1.0
    for mcol in range(64, 128):
        swp[mcol - 64, mcol] = 1.0
    swp = swp.astype(bf)
    idn = np.eye(128, dtype=np.float32).astype(bf)

    scale = 1.0 / math.sqrt(HD)
    in_maps = []
    for c in range(N_CORES):
        qcols = np.concatenate([(4 * c + hh) * HD + perm for hh in range(HL)])
        wq_c = (np.ascontiguousarray(wq[:, qcols]) * np.float32(scale)).astype(bf)
        wkv_c = np.ascontiguousarray(
            np.concatenate([wk[:, c * HD + perm], wv[:, c * HD:(c + 1) * HD]],
                           axis=1)).astype(bf)
        wo_c = wo[:, c * SQT:(c + 1) * SQT].astype(bf)
        # [D, SQT] -> [128, NK*SQT] partition-major (row g*128+p -> [p, g])
        wot_c = np.ascontiguousarray(
            wo_c.reshape(NK, 128, SQT).transpose(1, 0, 2).reshape(
                128, NK * SQT))
        in_maps.append({
            "xTt": xTt, "wqt": _tile_rows(wq_c), "wkvt": _tile_rows(wkv_c),
            "wot": wot_c,
            "cos2": cos2, "sin2": sin2, "mtd": mtd, "swp": swp, "idn": idn,
        })
    return in_maps


def kernel(x, wq, wk, wv, wo, cos, sin, mask, _trace=False):
    in_maps = _host_prep(np.asarray(x, np.float32), np.asarray(wq, np.float32),
                         np.asarray(wk, np.float32), np.asarray(wv, np.float32),
                         np.asarray(wo, np.float32), np.asarray(cos, np.float32),
                         np.asarray(sin, np.float32), np.asarray(mask, np.float32))
    if "nc" not in _CACHE:
        _CACHE["nc"] = build_nc()
    nc = _CACHE["nc"]
    res = run_bass_kernel_spmd(nc, in_maps, core_ids=list(range(N_CORES)),
                               trace=_trace,
                               trace_cores=list(range(N_CORES)) if _trace else None)
    out = np.empty((1, S, D), np.float32)
    for c in range(N_CORES):
        out[0, :, c * SQT:(c + 1) * SQT] = np.asarray(
            res.results[c]["out"], dtype=np.float32).T
    if _trace:
        _CACHE["last_exec_time_ns"] = res.exec_time_ns
        _CACHE["last_results"] = res
    return out



# revision 9
# speedup vs baseline: 1.0264x; 1.0264x over previous
"""GQA attention (B=1, S=2048, D=4096, HQ=32, HKV=8, HD=128) + RoPE + causal,
tensor-parallel over heads on 8 TRN2 NeuronCores.

v3 strategy (vs v2): the PE sustains ~1.95 GHz and weight loads are fully
hidden, so the wins are stall/ramp elimination, not fewer LDWEIGHTS:
 - causal mask applied as a cheap bf16 0/1 triu multiply AFTER exp (one
   [128,128] pattern shared by every diagonal block) instead of a f32
   PSUM-read mask add — removes ~50us of near-saturated DVE work.
 - projection split into a kv pass (2 PSUM banks) and a q pass (4 banks):
   kv(c+1) matmuls interleave into attention(c) heads 2-3 where the PE
   otherwise stalls on the exp chain; wo(0) interleaves into attention(3).
   PSUM banks are explicitly tag-partitioned (sps2/ops2/kv2/wo1/tmp1).
 - kv(0) starts ~3us in (needs only first x + wkv chunks; wq can arrive
   ~20us later for the q pass).
All PE matmuls bf16 (f32 PSUM)."""

import math

import ml_dtypes
import numpy as np

import concourse.bass as bass
import concourse.tile as tile
from concourse import bacc, mybir
from concourse.bass_utils import run_bass_kernel_spmd

F32 = mybir.dt.float32
BF16 = mybir.dt.bfloat16

S = 2048
D = 4096
HQ, HKV, HD = 32, 8, 128
HL = HQ // 8            # 4 local q heads per core
SQT = 512               # sq tile
NSQ = S // SQT          # 4
NK = D // 128           # 32 contraction k-tiles
NSK = S // 128          # 16 sk tiles
N_CORES = 8
AF = mybir.ActivationFunctionType
ALU = mybir.AluOpType


def dedup_ldweights(nc):
    """Remove InstLdweights whose weights AP identically matches the previous
    InstLdweights on the PE queue with only InstMatmult instructions between.
    (Measured neutral on HW — loads are already hidden — but trims the PE
    queue stream.)"""
    deleted = {}
    for f in nc.m.functions:
        for bb in f.blocks:
            insts = bb.instructions
            out = []
            last_key = None
            last_name = None
            pending_edges = []
            for i in insts:
                tn = type(i).__name__
                eng = getattr(i, "engine", None)
                is_pe = eng is not None and "PE" in str(eng)
                if tn == "InstLdweights":
                    key = str(i.ins[0])
                    if key == last_key:
                        deleted[i.name] = last_name
                        pending_edges.extend(i.dependency_edges())
                        continue
                    last_key = key
                    last_name = i.name
                elif tn == "InstMatmult":
                    if getattr(i, "is_transpose", False):
                        last_key = None
                    for nm, info in pending_edges:
                        i.add_dependency(nm, info)
                    pending_edges = []
                elif is_pe:
                    last_key = None
                out.append(i)
            assert not pending_edges
            bb.instructions = out
    if deleted:
        for f in nc.m.functions:
            for bb in f.blocks:
                for i in bb.instructions:
                    try:
                        i.remap_dependency_names(deleted)
                    except Exception:
                        pass
    return len(deleted)


def build_nc():
    nc = bacc.Bacc(num_devices=N_CORES, num_swdge_queues=4)

    # host-pre-tiled inputs: every DMA reads >=4KB contiguous per partition.
    xTt = nc.declare_dram_parameter("xTt", [NSQ * 8 * 128, 4 * SQT], BF16,
                                    isOutput=False)
    wqt = nc.declare_dram_parameter("wqt", [8 * 128, 4 * HL * HD], BF16,
                                    isOutput=False)
    wkvt = nc.declare_dram_parameter("wkvt", [8 * 128, 4 * 2 * HD], BF16,
                                     isOutput=False)
    wot = nc.declare_dram_parameter("wot", [128, NK * SQT], BF16,
                                    isOutput=False)  # col shard
    cos2 = nc.declare_dram_parameter("cos2", [128, S], BF16, isOutput=False)
    sin2 = nc.declare_dram_parameter("sin2", [128, S], BF16, isOutput=False)
    binm = nc.declare_dram_parameter("binm", [128, 128], BF16, isOutput=False)
    swp = nc.declare_dram_parameter("swp", [128, 128], BF16, isOutput=False)
    idn = nc.declare_dram_parameter("idn", [128, 128], BF16, isOutput=False)
    out = nc.declare_dram_parameter("out", [S, SQT], BF16, isOutput=True)

    with tile.TileContext(nc) as tc:
        with tc.tile_pool(name="const", bufs=1) as constp, \
             tc.tile_pool(name="wqp", bufs=3) as wqp, \
             tc.tile_pool(name="wkvp", bufs=1) as wkvp, \
             tc.tile_pool(name="wob", bufs=1) as wob, \
             tc.tile_pool(name="xp", bufs=8) as xp, \
             tc.tile_pool(name="qp", bufs=2) as qp, \
             tc.tile_pool(name="ep", bufs=6) as ep, \
             tc.tile_pool(name="eap", bufs=2) as eap, \
             tc.tile_pool(name="tp", bufs=2) as tp, \
             tc.tile_pool(name="agp", bufs=6) as agp, \
             tc.tile_pool(name="op", bufs=2) as opool, \
             tc.tile_pool(name="outp", bufs=2) as outp, \
             tc.tile_pool(name="ps", bufs=1, space="PSUM") as ps, \
             tc.tile_pool(name="dram", bufs=1, space="DRAM") as dramp:

            # PSUM bank partitioning (8 banks total), all [128,512] f32:
            #   sps(2): score tiles   ops(2): o_ps per head / q0,q1 accs
            #   kv (2): k,v accs / q2,q3 accs   wo(1): wo accumulator
            #   tmp(1): rope-swap temp, v-transpose temp, softmax-sum temp
            def pt(tag, name):
                bufs = {"sps": 2, "ops": 2, "kv": 2, "wo": 1, "tmp": 1}[tag]
                return ps.tile([128, SQT], F32, tag=tag, bufs=bufs, name=name)

            # ---- initial DMAs.  kv(0) needs only x chunk 0 + wkv chunk 0,
            # ---- so those lead their queues; wq (needed ~20us in at the q
            # ---- pass) streams behind x on gpsimd.
            wkv_t = [wkvp.tile([128, 4 * 2 * HD], BF16, tag=f"wkv{j}",
                               name=f"wkv{j}") for j in range(8)]
            xts = {}
            for j in range(8):
                xts[(0, j)] = xp.tile([128, 4 * SQT], BF16, tag="xt",
                                      name=f"x0{j}")
            for j in range(8):
                xeng = nc.gpsimd if j % 2 == 0 else nc.sync
                xeng.dma_start(out=xts[(0, j)],
                               in_=xTt[j * 128:(j + 1) * 128, :])
                nc.scalar.dma_start(out=wkv_t[j],
                                    in_=wkvt[j * 128:(j + 1) * 128, :])

            def wq_stream(c):
                # even chunks ride gpsimd (fast, behind x), odd chunks the
                # scalar hw queue so the stream lands in ~half the time
                ts = [wqp.tile([128, 4 * HL * HD], BF16, tag="wq",
                               name=f"wqs{c}{j}") for j in range(8)]
                for j in range(8):
                    eng = nc.gpsimd if j % 2 == 0 else nc.scalar
                    eng.dma_start(out=ts[j],
                                  in_=wqt[j * 128:(j + 1) * 128, :])
                return ts

            wqs = {0: wq_stream(0)}

            # constants on the scalar queue behind wkv
            cos_sb = constp.tile([128, S], BF16)
            nc.scalar.dma_start(out=cos_sb, in_=cos2[:, :])
            sin_sb = constp.tile([128, S], BF16)
            nc.scalar.dma_start(out=sin_sb, in_=sin2[:, :])
            bin_sb = constp.tile([128, 128], BF16)
            nc.scalar.dma_start(out=bin_sb, in_=binm[:, :])
            swp_sb = constp.tile([128, 128], BF16)
            nc.scalar.dma_start(out=swp_sb, in_=swp[:, :])
            idn_sb = constp.tile([128, 128], BF16)
            nc.scalar.dma_start(out=idn_sb, in_=idn[:, :])
            ones_f = constp.tile([128, 128], F32)
            nc.vector.memset(ones_f, 1.0)
            allones = constp.tile([128, 128], BF16)
            nc.scalar.activation(allones, ones_f, AF.Copy)

            kT_sb = constp.tile([128, S], BF16)      # kv head, feature-major
            v_sb = constp.tile([128, S], BF16)       # [sk%128, (sk//128)*128+d]

            wo_sb = wob.tile([128, NK * SQT], BF16)

            ag_in = [dramp.tile([HL * 128, SQT], BF16, name=f"agin{c}")
                     for c in range(NSQ)]
            ag_out = [dramp.tile([HQ * 128, SQT], BF16, addr_space="Shared",
                                 name=f"agout{c}") for c in range(NSQ)]

            def rope(ps_t, cos_cols, sin_cols, dst):
                """dst[bf16 128xSQT] = cos*ps + signed-half-swap(sin*ps)."""
                t2 = tp.tile([128, SQT], BF16, tag="t2")
                nc.vector.tensor_tensor(out=t2, in0=ps_t, in1=sin_cols,
                                        op=ALU.mult)
                t2s = pt("tmp", "t2s")
                nc.tensor.matmul(t2s, swp_sb, t2, start=True, stop=True)
                t1 = tp.tile([128, SQT], BF16, tag="t1")
                nc.vector.tensor_tensor(out=t1, in0=ps_t, in1=cos_cols,
                                        op=ALU.mult)
                nc.vector.tensor_tensor(out=dst, in0=t1, in1=t2s, op=ALU.add)

            # ---- filler streams: emit_filler(n) emits up to n deferred PE
            # ---- matmuls (kv proj of the next tile, or wo(0) during attn(3))
            filler = []

            def emit_filler(n):
                for _ in range(n):
                    if not filler:
                        return
                    filler.pop(0)()

            def kv_proj_ops(c):
                """Return list of closures: kv projection matmuls for tile c
                (2 PSUM banks) + the trailing v drain."""
                k_ps = pt("kv", f"kps{c}")
                v_ps = pt("kv", f"vps{c}")
                ops = []
                for kt in range(NK):
                    t = kt % 4

                    def mm(kt=kt, t=t):
                        xt = xts[(c, kt // 4)][:, t * SQT:(t + 1) * SQT]
                        nc.tensor.matmul(
                            k_ps,
                            wkv_t[kt // 4][:, t * 2 * HD:t * 2 * HD + HD],
                            xt, start=(kt == 0), stop=(kt == NK - 1))
                        nc.tensor.matmul(
                            v_ps,
                            wkv_t[kt // 4][:, t * 2 * HD + HD:(t + 1) * 2 * HD],
                            xt, start=(kt == 0), stop=(kt == NK - 1))
                    ops.append(mm)
                return ops, k_ps, v_ps

            def q_phase(c, k_ps, v_ps):
                """rope k, transpose v, project+rope q0/q1 (q2/q3 roped lazily
                during attention).  Returns (qT_sb, q_ps list)."""
                s0 = c * SQT
                cse = (slice(None), slice(s0, s0 + SQT))
                vt_sb = tp.tile([128, SQT], BF16, tag="vt", bufs=2)
                nc.scalar.activation(vt_sb, v_ps, AF.Copy)
                rope(k_ps, cos_sb[cse], sin_sb[cse], kT_sb[cse])
                for sb in range(SQT // 128):
                    vp = ps.tile([128, 128], BF16, tag="tmp", bufs=1,
                                 name="vtp")
                    nc.tensor.transpose(vp, vt_sb[:, sb * 128:(sb + 1) * 128],
                                        idn_sb)
                    nc.scalar.activation(
                        v_sb[:, (4 * c + sb) * 128:(4 * c + sb + 1) * 128],
                        vp, AF.Copy)
                q_ps = [pt("ops", f"qps{c}0"), pt("ops", f"qps{c}1"),
                        pt("kv", f"qps{c}2"), pt("kv", f"qps{c}3")]
                for kt in range(NK):
                    t = kt % 4
                    xt = xts[(c, kt // 4)][:, t * SQT:(t + 1) * SQT]
                    for h in range(HL):
                        nc.tensor.matmul(
                            q_ps[h],
                            wqs[c][kt // 4][:, t * HL * HD + h * 128:
                                            t * HL * HD + (h + 1) * 128],
                            xt, start=(kt == 0), stop=(kt == NK - 1))
                qT_sb = qp.tile([128, HL, SQT], BF16, tag="qT")
                rope(q_ps[0], cos_sb[cse], sin_sb[cse], qT_sb[:, 0, :])
                rope(q_ps[1], cos_sb[cse], sin_sb[cse], qT_sb[:, 1, :])
                return qT_sb, q_ps

            def attention_head(c, h, qT_sb, fin_prev):
                """One head's scores/exp/PV; returns a finalize closure the
                caller emits later.  Diagonal blocks: exp on raw scores then a
                bf16 0/1 triu multiply (same [128,128] pattern every block).
                e accumulated f32, split DVE (even blocks) / GpSimd (odd)."""
                nsk_here = 4 * c + 4
                o_ps = pt("ops", f"o{c}{h}")
                e_acc_d = eap.tile([128, SQT], F32, tag="eaccd")
                e_acc_g = eap.tile([128, SQT], F32, tag="eaccg")
                if c == 0:
                    nc.gpsimd.memset(e_acc_g, 0.0)

                def flush(pending):
                    pe, pc, pk = pending
                    nc.tensor.matmul(
                        o_ps[:, pc:], v_sb[:, pk * 128:(pk + 1) * 128],
                        pe[:, pc:], start=(pk == 0),
                        stop=(pk == nsk_here - 1))

                pending = []  # (e_sb, col0, kt2) awaiting PV
                for kt2 in range(nsk_here):
                    m = kt2 - 4 * c
                    col0 = 128 * m if m > 0 else 0
                    s_ps = pt("sps", "s_ps")
                    nc.tensor.matmul(
                        s_ps[:, col0:], kT_sb[:, kt2 * 128:(kt2 + 1) * 128],
                        qT_sb[:, h, col0:], start=True, stop=True)
                    e_sb = ep.tile([128, SQT], BF16, tag="e")
                    nc.scalar.activation(e_sb[:, col0:], s_ps[:, col0:],
                                         AF.Exp)
                    if m >= 0:
                        # zero the strictly-upper triangle of the diagonal
                        # 128-col strip (bf16 mult, ~4x cheaper than the f32
                        # PSUM mask add it replaces)
                        nc.vector.tensor_tensor(
                            out=e_sb[:, col0:col0 + 128],
                            in0=e_sb[:, col0:col0 + 128],
                            in1=bin_sb, op=ALU.mult)
                    eng = nc.vector if kt2 % 2 == 0 else nc.gpsimd
                    acc = e_acc_d if kt2 % 2 == 0 else e_acc_g
                    if kt2 == 0 or (kt2 == 1 and c > 0):
                        eng.tensor_copy(out=acc, in_=e_sb)
                    else:
                        eng.tensor_tensor(out=acc[:, col0:],
                                          in0=acc[:, col0:],
                                          in1=e_sb[:, col0:], op=ALU.add)
                    pending.append((e_sb, col0, kt2))
                    if len(pending) > 3:
                        flush(pending.pop(0))
                    if kt2 == 2 and fin_prev is not None:
                        fin_prev()
                    # kv fillers touch the kv PSUM slots, free only once
                    # q2/q3 are roped (ends of heads 0/1); wo fillers (last
                    # tile) have no such hazard and can start at head 0
                    if h >= 2 or c == NSQ - 1:
                        emit_filler(2 if c < 2 else 4)
                for p in pending:
                    flush(p)

                def finalize():
                    ea_bf = eap.tile([128, SQT], BF16, tag="eaccb")
                    nc.vector.tensor_tensor(out=ea_bf, in0=e_acc_d,
                                            in1=e_acc_g, op=ALU.add)
                    sum_ps = pt("tmp", f"sb{c}{h}")
                    nc.tensor.matmul(sum_ps, allones, ea_bf,
                                     start=True, stop=True)
                    rec_sb = opool.tile([128, SQT], F32, tag="rcb")
                    nc.vector.reciprocal_approx_fast(out=rec_sb, in_=sum_ps)
                    on_sb = opool.tile([128, SQT], BF16, tag="on")
                    nc.vector.tensor_tensor(out=on_sb, in0=rec_sb, in1=o_ps,
                                            op=ALU.mult)
                    nc.gpsimd.dma_start(
                        out=ag_in[c][h * 128:(h + 1) * 128, :], in_=on_sb)
                return finalize

            def wo_ops(c):
                """Closures for wo(c): 4 row-blocks x 32 contraction matmuls
                on the single 'wo' PSUM bank, drained per block."""
                ag_ts = []
                for mt in range(4):
                    ag_t = agp.tile([128, HQ, 128], BF16, tag="ag",
                                    name=f"agt{c}{mt}")
                    nc.sync.dma_start(
                        out=ag_t,
                        in_=ag_out[c][:, mt * 128:(mt + 1) * 128].rearrange(
                            "(g p) n -> p g n", p=128))
                    ag_ts.append(ag_t)
                ops = []
                state = {}
                for mt in range(4):
                    def alloc(mt=mt):
                        state["o1"] = pt("wo", f"wops{c}{mt}")
                    for g in range(NK):
                        def mm(mt=mt, g=g):
                            if g == 0:
                                alloc(mt)
                            nc.tensor.matmul(
                                state["o1"], ag_ts[mt][:, g, :],
                                wo_sb[:, g * SQT:(g + 1) * SQT],
                                start=(g == 0), stop=(g == NK - 1))
                            if g == NK - 1:
                                ob = outp.tile([128, SQT], BF16, tag="ob")
                                nc.scalar.activation(ob, state["o1"], AF.Copy)
                                nc.scalar.dma_start(
                                    out=out[c * SQT + mt * 128:
                                            c * SQT + (mt + 1) * 128, :],
                                    in_=ob)
                        ops.append(mm)
                return ops

            # ================= main schedule =================
            kv_ops, k_ps, v_ps = kv_proj_ops(0)
            for op_ in kv_ops:
                op_()

            for c in range(NSQ):
                s0 = c * SQT
                cse = (slice(None), slice(s0, s0 + SQT))
                qT_sb, q_ps = q_phase(c, k_ps, v_ps)

                # prefetch next tile's x (consumed by interleaved kv proj)
                # and wq (needed at q_phase(c+1))
                if c + 1 < NSQ:
                    for j in range(8):
                        xts[(c + 1, j)] = xp.tile([128, 4 * SQT], BF16,
                                                  tag="xt", name=f"x{c+1}{j}")
                        xeng = nc.gpsimd if j % 2 == 0 else nc.sync
                        xeng.dma_start(
                            out=xts[(c + 1, j)],
                            in_=xTt[((c + 1) * 8 + j) * 128:
                                    ((c + 1) * 8 + j + 1) * 128, :])
                    wqs[c + 1] = wq_stream(c + 1)
                    kv_ops, k_ps, v_ps = kv_proj_ops(c + 1)
                    filler.extend(kv_ops)
                if c == 1:
                    nc.gpsimd.dma_start(out=wo_sb, in_=wot[:, :])
                if c == NSQ - 1:
                    filler.extend(wo_ops(0))

                fin = None
                for h in range(HL):
                    fin = attention_head(c, h, qT_sb, fin)
                    if h + 2 < HL:
                        rope(q_ps[h + 2], cos_sb[cse], sin_sb[cse],
                             qT_sb[:, h + 2, :])
                fin()
                emit_filler(len(filler))  # drain leftover before AG

                nc.gpsimd.collective_compute(
                    "AllGather", ALU.bypass,
                    replica_groups=[list(range(N_CORES))],
                    ins=[ag_in[c].opt()], outs=[ag_out[c].opt()])

            # tail: remaining wo phases (wo(0) ran inside attention(3))
            for cw in range(1, NSQ):
                for op_ in wo_ops(cw):
                    op_()

    dedup_ldweights(nc)
    nc.finalize()
    return nc


_CACHE = {}


def _tile_rows(a, chunk=4):
    """[D, C] -> [D//(128*chunk) * 128, chunk*C]: row blocks partition-major
    so each DMA partition reads chunk*C contiguous elements."""
    dd, cc = a.shape
    nj = dd // (128 * chunk)
    return np.ascontiguousarray(
        a.reshape(nj, chunk, 128, cc).transpose(0, 2, 1, 3).reshape(
            nj * 128, chunk * cc))


def _host_prep(x, wq, wk, wv, wo, cos, sin, mask):
    perm = np.concatenate([np.arange(0, HD, 2), np.arange(1, HD, 2)])
    bf = ml_dtypes.bfloat16
    xT = np.ascontiguousarray(x.reshape(S, D).T).astype(bf)
    xTt = np.concatenate(
        [_tile_rows(xT[:, c * SQT:(c + 1) * SQT]) for c in range(NSQ)],
        axis=0)
    cos2 = np.ascontiguousarray(np.vstack([cos.T, cos.T])).astype(bf)
    sin2 = np.ascontiguousarray(np.vstack([sin.T, sin.T])).astype(bf)
    # 0/1 mask for the diagonal 128-strip: allowed iff col >= partition
    binm = np.triu(np.ones((128, 128), np.float32)).astype(bf)
    swp = np.zeros((128, 128), np.float32)
    for mcol in range(64):
        swp[mcol + 64, mcol] = -1.0
    for mcol in range(64, 128):
        swp[mcol - 64, mcol] = 1.0
    swp = swp.astype(bf)
    idn = np.eye(128, dtype=np.float32).astype(bf)

    scale = 1.0 / math.sqrt(HD)
    in_maps = []
    for c in range(N_CORES):
        qcols = np.concatenate([(4 * c + hh) * HD + perm for hh in range(HL)])
        wq_c = (np.ascontiguousarray(wq[:, qcols]) * np.float32(scale)).astype(bf)
        wkv_c = np.ascontiguousarray(
            np.concatenate([wk[:, c * HD + perm], wv[:, c * HD:(c + 1) * HD]],
                           axis=1)).astype(bf)
        wo_c = wo[:, c * SQT:(c + 1) * SQT].astype(bf)
        # [D, SQT] -> [128, NK*SQT] partition-major (row g*128+p -> [p, g])
        wot_c = np.ascontiguousarray(
            wo_c.reshape(NK, 128, SQT).transpose(1, 0, 2).reshape(
                128, NK * SQT))
        in_maps.append({
            "xTt": xTt, "wqt": _tile_rows(wq_c), "wkvt": _tile_rows(wkv_c),
            "wot": wot_c,
            "cos2": cos2, "sin2": sin2, "binm": binm, "swp": swp, "idn": idn,
        })
    return in_maps


def kernel(x, wq, wk, wv, wo, cos, sin, mask, _trace=False):
    in_maps = _host_prep(np.asarray(x, np.float32), np.asarray(wq, np.float32),
                         np.asarray(wk, np.float32), np.asarray(wv, np.float32),
                         np.asarray(wo, np.float32), np.asarray(cos, np.float32),
                         np.asarray(sin, np.float32), np.asarray(mask, np.float32))
    if "nc" not in _CACHE:
        _CACHE["nc"] = build_nc()
    nc = _CACHE["nc"]
    res = run_bass_kernel_spmd(nc, in_maps, core_ids=list(range(N_CORES)),
                               trace=_trace,
                               trace_cores=list(range(N_CORES)) if _trace else None)
    out = np.empty((1, S, D), np.float32)
    for c in range(N_CORES):
        out[0, :, c * SQT:(c + 1) * SQT] = np.asarray(
            res.results[c]["out"], dtype=np.float32)
    if _trace:
        _CACHE["last_exec_time_ns"] = res.exec_time_ns
        _CACHE["last_results"] = res
    return out


# revision 14
# speedup vs baseline: 1.0317x; 1.0051x over previous
"""GQA attention (B=1, S=2048, D=4096, HQ=32, HKV=8, HD=128) + RoPE + causal,
tensor-parallel over heads on 8 TRN2 NeuronCores.

v3 strategy (vs v2): the PE sustains ~1.95 GHz and weight loads are fully
hidden, so the wins are stall/ramp elimination, not fewer LDWEIGHTS:
 - causal mask applied as a cheap bf16 0/1 triu multiply AFTER exp (one
   [128,128] pattern shared by every diagonal block) instead of a f32
   PSUM-read mask add — removes ~50us of near-saturated DVE work.
 - projection split into a kv pass (2 PSUM banks) and a q pass (4 banks):
   kv(c+1) matmuls interleave into attention(c) heads 2-3 where the PE
   otherwise stalls on the exp chain; wo(0) interleaves into attention(3).
   PSUM banks are explicitly tag-partitioned (sps2/ops2/kv2/wo1/tmp1).
 - kv(0) starts ~3us in (needs only first x + wkv chunks; wq can arrive
   ~20us later for the q pass).
All PE matmuls bf16 (f32 PSUM)."""

import math

import ml_dtypes
import numpy as np

import concourse.bass as bass
import concourse.tile as tile
from concourse import bacc, mybir
from concourse.bass_utils import run_bass_kernel_spmd

F32 = mybir.dt.float32
BF16 = mybir.dt.bfloat16

S = 2048
D = 4096
HQ, HKV, HD = 32, 8, 128
HL = HQ // 8            # 4 local q heads per core
SQT = 512               # sq tile
NSQ = S // SQT          # 4
NK = D // 128           # 32 contraction k-tiles
NSK = S // 128          # 16 sk tiles
N_CORES = 8
AF = mybir.ActivationFunctionType
ALU = mybir.AluOpType


def dedup_ldweights(nc):
    """Remove InstLdweights whose weights AP identically matches the previous
    InstLdweights on the PE queue with only InstMatmult instructions between.
    (Measured neutral on HW — loads are already hidden — but trims the PE
    queue stream.)"""
    deleted = {}
    for f in nc.m.functions:
        for bb in f.blocks:
            insts = bb.instructions
            out = []
            last_key = None
            last_name = None
            pending_edges = []
            for i in insts:
                tn = type(i).__name__
                eng = getattr(i, "engine", None)
                is_pe = eng is not None and "PE" in str(eng)
                if tn == "InstLdweights":
                    key = str(i.ins[0])
                    if key == last_key:
                        deleted[i.name] = last_name
                        pending_edges.extend(i.dependency_edges())
                        continue
                    last_key = key
                    last_name = i.name
                elif tn == "InstMatmult":
                    if getattr(i, "is_transpose", False):
                        last_key = None
                    for nm, info in pending_edges:
                        i.add_dependency(nm, info)
                    pending_edges = []
                elif is_pe:
                    last_key = None
                out.append(i)
            assert not pending_edges
            bb.instructions = out
    if deleted:
        for f in nc.m.functions:
            for bb in f.blocks:
                for i in bb.instructions:
                    try:
                        i.remap_dependency_names(deleted)
                    except Exception:
                        pass
    return len(deleted)


def build_nc():
    nc = bacc.Bacc(num_devices=N_CORES, num_swdge_queues=4)

    # host-pre-tiled inputs: every DMA reads >=4KB contiguous per partition.
    xTt = nc.declare_dram_parameter("xTt", [NSQ * 8 * 128, 4 * SQT], BF16,
                                    isOutput=False)
    wqt = nc.declare_dram_parameter("wqt", [8 * 128, 4 * HL * HD], BF16,
                                    isOutput=False)
    wkvt = nc.declare_dram_parameter("wkvt", [8 * 128, 4 * 2 * HD], BF16,
                                     isOutput=False)
    wot = nc.declare_dram_parameter("wot", [128, NK * SQT], BF16,
                                    isOutput=False)  # col shard
    cos2 = nc.declare_dram_parameter("cos2", [128, S], BF16, isOutput=False)
    sin2 = nc.declare_dram_parameter("sin2", [128, S], BF16, isOutput=False)
    binm = nc.declare_dram_parameter("binm", [128, 128], BF16, isOutput=False)
    swp = nc.declare_dram_parameter("swp", [128, 128], BF16, isOutput=False)
    idn = nc.declare_dram_parameter("idn", [128, 128], BF16, isOutput=False)
    out = nc.declare_dram_parameter("out", [S, SQT], BF16, isOutput=True)

    with tile.TileContext(nc) as tc:
        with tc.tile_pool(name="const", bufs=1) as constp, \
             tc.tile_pool(name="wqp", bufs=3) as wqp, \
             tc.tile_pool(name="wkvp", bufs=1) as wkvp, \
             tc.tile_pool(name="wob", bufs=1) as wob, \
             tc.tile_pool(name="xp", bufs=8) as xp, \
             tc.tile_pool(name="qp", bufs=2) as qp, \
             tc.tile_pool(name="ep", bufs=6) as ep, \
             tc.tile_pool(name="eap", bufs=2) as eap, \
             tc.tile_pool(name="tp", bufs=2) as tp, \
             tc.tile_pool(name="agp", bufs=6) as agp, \
             tc.tile_pool(name="op", bufs=2) as opool, \
             tc.tile_pool(name="outp", bufs=2) as outp, \
             tc.tile_pool(name="ps", bufs=1, space="PSUM") as ps, \
             tc.tile_pool(name="dram", bufs=1, space="DRAM") as dramp:

            # PSUM bank partitioning (8 banks total), all [128,512] f32:
            #   sps(2): score tiles   ops(2): o_ps per head / q0,q1 accs
            #   kv (2): k,v accs / q2,q3 accs   wo(1): wo accumulator
            #   tmp(1): rope-swap temp, v-transpose temp, softmax-sum temp
            def pt(tag, name):
                bufs = {"sps": 2, "ops": 2, "kv": 2, "wo": 1, "tmp": 1}[tag]
                return ps.tile([128, SQT], F32, tag=tag, bufs=bufs, name=name)

            # ---- initial DMAs.  The kv(0) pass consumes x+wkv chunks at
            # ---- ~235 GB/s, so the first tile's feed is spread across all
            # ---- three queues interleaved in consumption order.
            wkv_t = [wkvp.tile([128, 4 * 2 * HD], BF16, tag=f"wkv{j}",
                               name=f"wkv{j}") for j in range(8)]
            xts = {}
            for j in range(8):
                xts[(0, j)] = xp.tile([128, 4 * SQT], BF16, tag="xt",
                                      name=f"x0{j}")

            def x0dma(j, eng):
                eng.dma_start(out=xts[(0, j)],
                              in_=xTt[j * 128:(j + 1) * 128, :])

            def wkvdma(j, eng):
                eng.dma_start(out=wkv_t[j], in_=wkvt[j * 128:(j + 1) * 128, :])

            for j, eng in ((0, nc.gpsimd), (2, nc.gpsimd), (4, nc.gpsimd),
                           (6, nc.gpsimd)):
                x0dma(j, eng)
            wkvdma(0, nc.scalar)
            wkvdma(1, nc.sync)
            x0dma(1, nc.sync)
            wkvdma(2, nc.scalar)
            wkvdma(3, nc.sync)
            x0dma(3, nc.scalar)
            wkvdma(4, nc.scalar)
            wkvdma(5, nc.sync)
            x0dma(5, nc.sync)
            wkvdma(6, nc.scalar)
            wkvdma(7, nc.sync)
            x0dma(7, nc.scalar)

            def wq_stream(c):
                # even chunks ride gpsimd (fast, behind x), odd chunks the
                # scalar hw queue so the stream lands in ~half the time
                ts = [wqp.tile([128, 4 * HL * HD], BF16, tag="wq",
                               name=f"wqs{c}{j}") for j in range(8)]
                for j in range(8):
                    eng = nc.gpsimd if j % 2 == 0 else nc.scalar
                    eng.dma_start(out=ts[j],
                                  in_=wqt[j * 128:(j + 1) * 128, :])
                return ts

            wqs = {0: wq_stream(0)}

            # constants ride gpsimd behind the x evens: rope(k0) needs
            # cos/sin/swp at ~25us, before the scalar/sync queues drain
            cos_sb = constp.tile([128, S], BF16)
            nc.gpsimd.dma_start(out=cos_sb, in_=cos2[:, :])
            sin_sb = constp.tile([128, S], BF16)
            nc.gpsimd.dma_start(out=sin_sb, in_=sin2[:, :])
            swp_sb = constp.tile([128, 128], BF16)
            nc.gpsimd.dma_start(out=swp_sb, in_=swp[:, :])
            bin_sb = constp.tile([128, 128], BF16)
            nc.gpsimd.dma_start(out=bin_sb, in_=binm[:, :])
            idn_sb = constp.tile([128, 128], BF16)
            nc.gpsimd.dma_start(out=idn_sb, in_=idn[:, :])
            ones_f = constp.tile([128, 128], F32)
            nc.vector.memset(ones_f, 1.0)
            allones = constp.tile([128, 128], BF16)
            nc.scalar.activation(allones, ones_f, AF.Copy)

            kT_sb = constp.tile([128, S], BF16)      # kv head, feature-major
            v_sb = constp.tile([128, S], BF16)       # [sk%128, (sk//128)*128+d]

            wo_sb = wob.tile([128, NK * SQT], BF16)

            ag_in = [dramp.tile([HL * 128, SQT], BF16, name=f"agin{c}")
                     for c in range(NSQ)]
            ag_out = [dramp.tile([HQ * 128, SQT], BF16, addr_space="Shared",
                                 name=f"agout{c}") for c in range(NSQ)]

            def rope(ps_t, cos_cols, sin_cols, dst):
                """dst[bf16 128xSQT] = cos*ps + signed-half-swap(sin*ps)."""
                t2 = tp.tile([128, SQT], BF16, tag="t2")
                nc.vector.tensor_tensor(out=t2, in0=ps_t, in1=sin_cols,
                                        op=ALU.mult)
                t2s = pt("tmp", "t2s")
                nc.tensor.matmul(t2s, swp_sb, t2, start=True, stop=True)
                t1 = tp.tile([128, SQT], BF16, tag="t1")
                nc.vector.tensor_tensor(out=t1, in0=ps_t, in1=cos_cols,
                                        op=ALU.mult)
                nc.vector.tensor_tensor(out=dst, in0=t1, in1=t2s, op=ALU.add)

            # ---- filler streams: emit_filler(n) emits up to n deferred PE
            # ---- matmuls (kv proj of the next tile, or wo(0) during attn(3))
            filler = []

            def emit_filler(n):
                for _ in range(n):
                    if not filler:
                        return
                    filler.pop(0)()

            def kv_proj_ops(c):
                """Return list of closures: kv projection matmuls for tile c
                (2 PSUM banks) + the trailing v drain."""
                k_ps = pt("kv", f"kps{c}")
                v_ps = pt("kv", f"vps{c}")
                ops = []
                for kt in range(NK):
                    t = kt % 4

                    def mm(kt=kt, t=t):
                        xt = xts[(c, kt // 4)][:, t * SQT:(t + 1) * SQT]
                        nc.tensor.matmul(
                            k_ps,
                            wkv_t[kt // 4][:, t * 2 * HD:t * 2 * HD + HD],
                            xt, start=(kt == 0), stop=(kt == NK - 1))
                        nc.tensor.matmul(
                            v_ps,
                            wkv_t[kt // 4][:, t * 2 * HD + HD:(t + 1) * 2 * HD],
                            xt, start=(kt == 0), stop=(kt == NK - 1))
                    ops.append(mm)
                return ops, k_ps, v_ps

            def q_phase(c, k_ps, v_ps):
                """rope k, transpose v, project+rope q0/q1 (q2/q3 roped lazily
                during attention).  Returns (qT_sb, q_ps list)."""
                s0 = c * SQT
                cse = (slice(None), slice(s0, s0 + SQT))
                vt_sb = tp.tile([128, SQT], BF16, tag="vt", bufs=2)
                nc.scalar.activation(vt_sb, v_ps, AF.Copy)
                rope(k_ps, cos_sb[cse], sin_sb[cse], kT_sb[cse])
                for sb in range(SQT // 128):
                    vp = ps.tile([128, 128], BF16, tag="tmp", bufs=1,
                                 name="vtp")
                    nc.tensor.transpose(vp, vt_sb[:, sb * 128:(sb + 1) * 128],
                                        idn_sb)
                    nc.scalar.activation(
                        v_sb[:, (4 * c + sb) * 128:(4 * c + sb + 1) * 128],
                        vp, AF.Copy)
                q_ps = [pt("ops", f"qps{c}0"), pt("ops", f"qps{c}1"),
                        pt("kv", f"qps{c}2"), pt("kv", f"qps{c}3")]
                for kt in range(NK):
                    t = kt % 4
                    xt = xts[(c, kt // 4)][:, t * SQT:(t + 1) * SQT]
                    for h in range(HL):
                        nc.tensor.matmul(
                            q_ps[h],
                            wqs[c][kt // 4][:, t * HL * HD + h * 128:
                                            t * HL * HD + (h + 1) * 128],
                            xt, start=(kt == 0), stop=(kt == NK - 1))
                qT_sb = qp.tile([128, HL, SQT], BF16, tag="qT")
                rope(q_ps[0], cos_sb[cse], sin_sb[cse], qT_sb[:, 0, :])
                rope(q_ps[1], cos_sb[cse], sin_sb[cse], qT_sb[:, 1, :])
                return qT_sb, q_ps

            def attention_head(c, h, qT_sb, fin_prev):
                """One head's scores/exp/PV; returns a finalize closure the
                caller emits later.  Diagonal blocks: exp on raw scores then a
                bf16 0/1 triu multiply (same [128,128] pattern every block).
                e accumulated f32, split DVE (even blocks) / GpSimd (odd)."""
                nsk_here = 4 * c + 4
                o_ps = pt("ops", f"o{c}{h}")
                e_acc_d = eap.tile([128, SQT], F32, tag="eaccd")
                e_acc_g = eap.tile([128, SQT], F32, tag="eaccg")
                if c == 0:
                    nc.gpsimd.memset(e_acc_g, 0.0)

                def flush(pending):
                    pe, pc, pk = pending
                    nc.tensor.matmul(
                        o_ps[:, pc:], v_sb[:, pk * 128:(pk + 1) * 128],
                        pe[:, pc:], start=(pk == 0),
                        stop=(pk == nsk_here - 1))

                pending = []  # (e_sb, col0, kt2) awaiting PV
                for kt2 in range(nsk_here):
                    m = kt2 - 4 * c
                    col0 = 128 * m if m > 0 else 0
                    s_ps = pt("sps", "s_ps")
                    nc.tensor.matmul(
                        s_ps[:, col0:], kT_sb[:, kt2 * 128:(kt2 + 1) * 128],
                        qT_sb[:, h, col0:], start=True, stop=True)
                    e_sb = ep.tile([128, SQT], BF16, tag="e")
                    nc.scalar.activation(e_sb[:, col0:], s_ps[:, col0:],
                                         AF.Exp)
                    if m >= 0:
                        # zero the strictly-upper triangle of the diagonal
                        # 128-col strip (bf16 mult, ~4x cheaper than the f32
                        # PSUM mask add it replaces)
                        nc.vector.tensor_tensor(
                            out=e_sb[:, col0:col0 + 128],
                            in0=e_sb[:, col0:col0 + 128],
                            in1=bin_sb, op=ALU.mult)
                    eng = nc.vector if kt2 % 2 == 0 else nc.gpsimd
                    acc = e_acc_d if kt2 % 2 == 0 else e_acc_g
                    if kt2 == 0 or (kt2 == 1 and c > 0):
                        eng.tensor_copy(out=acc, in_=e_sb)
                    else:
                        eng.tensor_tensor(out=acc[:, col0:],
                                          in0=acc[:, col0:],
                                          in1=e_sb[:, col0:], op=ALU.add)
                    pending.append((e_sb, col0, kt2))
                    if len(pending) > 3:
                        flush(pending.pop(0))
                    if kt2 == 2 and fin_prev is not None:
                        fin_prev()
                    # kv fillers touch the kv PSUM slots, free only once
                    # q2/q3 are roped (ends of heads 0/1); wo fillers (last
                    # tile) have no such hazard and can start at head 0
                    if h >= 2 or c == NSQ - 1:
                        emit_filler(2 if c < 2 else 4)
                for p in pending:
                    flush(p)

                def finalize():
                    ea_bf = eap.tile([128, SQT], BF16, tag="eaccb")
                    nc.vector.tensor_tensor(out=ea_bf, in0=e_acc_d,
                                            in1=e_acc_g, op=ALU.add)
                    sum_ps = pt("tmp", f"sb{c}{h}")
                    nc.tensor.matmul(sum_ps, allones, ea_bf,
                                     start=True, stop=True)
                    rec_sb = opool.tile([128, SQT], F32, tag="rcb")
                    nc.vector.reciprocal_approx_fast(out=rec_sb, in_=sum_ps)
                    on_sb = opool.tile([128, SQT], BF16, tag="on")
                    nc.vector.tensor_tensor(out=on_sb, in0=rec_sb, in1=o_ps,
                                            op=ALU.mult)
                    nc.gpsimd.dma_start(
                        out=ag_in[c][h * 128:(h + 1) * 128, :], in_=on_sb)
                return finalize

            def wo_ops(c, tag, wait_ms):
                """Closures for wo(c): 4 row-blocks x 32 contraction matmuls.
                tile_wait_until pins the fetches AND matmuls at/after the sim
                time their AllGather is genuinely done — the scheduler's sim
                treats collectives as instant and otherwise hoists these into
                earlier phases, where on real HW they block the in-order PE
                queue on AG completion (~30us skew) [seen: 32us stall]."""
                ag_ts = []
                for mt in range(4):
                    ag_t = agp.tile([128, HQ, 128], BF16, tag="ag",
                                    name=f"agt{c}{mt}")
                    eng = nc.sync if mt % 2 == 0 else nc.scalar
                    with tc.tile_wait_until(wait_ms):
                        eng.dma_start(
                            out=ag_t,
                            in_=ag_out[c][:, mt * 128:(mt + 1) * 128]
                            .rearrange("(g p) n -> p g n", p=128))
                    ag_ts.append(ag_t)
                ops = []
                state = {}
                for mt in range(4):
                    for g in range(NK):
                        def mm(mt=mt, g=g):
                            with tc.tile_wait_until(wait_ms + 0.01):
                                if g == 0:
                                    state["o1"] = pt(tag, f"wops{c}{mt}")
                                nc.tensor.matmul(
                                    state["o1"], ag_ts[mt][:, g, :],
                                    wo_sb[:, g * SQT:(g + 1) * SQT],
                                    start=(g == 0), stop=(g == NK - 1))
                                if g == NK - 1:
                                    ob = outp.tile([128, SQT], BF16, tag="ob")
                                    nc.scalar.activation(ob, state["o1"],
                                                         AF.Copy)
                                    nc.scalar.dma_start(
                                        out=out[c * SQT + mt * 128:
                                                c * SQT + (mt + 1) * 128, :],
                                        in_=ob)
                        ops.append(mm)
                return ops

            # ================= main schedule =================
            kv_ops, k_ps, v_ps = kv_proj_ops(0)
            for op_ in kv_ops:
                op_()

            for c in range(NSQ):
                s0 = c * SQT
                cse = (slice(None), slice(s0, s0 + SQT))
                qT_sb, q_ps = q_phase(c, k_ps, v_ps)

                # prefetch next tile's x (consumed by interleaved kv proj)
                # and wq (needed at q_phase(c+1))
                if c + 1 < NSQ:
                    for j in range(8):
                        xts[(c + 1, j)] = xp.tile([128, 4 * SQT], BF16,
                                                  tag="xt", name=f"x{c+1}{j}")
                        xeng = nc.gpsimd if j % 2 == 0 else nc.sync
                        xeng.dma_start(
                            out=xts[(c + 1, j)],
                            in_=xTt[((c + 1) * 8 + j) * 128:
                                    ((c + 1) * 8 + j + 1) * 128, :])
                    wqs[c + 1] = wq_stream(c + 1)
                    kv_ops, k_ps, v_ps = kv_proj_ops(c + 1)
                    filler.extend(kv_ops)
                if c == 1:
                    nc.gpsimd.dma_start(out=wo_sb, in_=wot[:, :])
                if c == NSQ - 1:
                    filler.extend(wo_ops(0, "wo", 0.19))

                fin = None
                for h in range(HL):
                    fin = attention_head(c, h, qT_sb, fin)
                    if h + 2 < HL:
                        rope(q_ps[h + 2], cos_sb[cse], sin_sb[cse],
                             qT_sb[:, h + 2, :])
                fin()
                emit_filler(len(filler))  # drain leftover before AG

                nc.gpsimd.collective_compute(
                    "AllGather", ALU.bypass,
                    replica_groups=[list(range(N_CORES))],
                    ins=[ag_in[c].opt()], outs=[ag_out[c].opt()])

            # tail: remaining wo phases (wo(0) ran inside attention(3)).
            # Accumulators ride the 'kv' ring (idle after attention(3)) so
            # consecutive row-blocks pipeline 2-deep across the ACT drain.
            for cw, wms in ((1, 0.26), (2, 0.29), (3, 0.32)):
                for op_ in wo_ops(cw, "kv", wms):
                    op_()

    dedup_ldweights(nc)
    nc.finalize()
    return nc


_CACHE = {}


def _tile_rows(a, chunk=4):
    """[D, C] -> [D//(128*chunk) * 128, chunk*C]: row blocks partition-major
    so each DMA partition reads chunk*C contiguous elements."""
    dd, cc = a.shape
    nj = dd // (128 * chunk)
    return np.ascontiguousarray(
        a.reshape(nj, chunk, 128, cc).transpose(0, 2, 1, 3).reshape(
            nj * 128, chunk * cc))


def _host_prep(x, wq, wk, wv, wo, cos, sin, mask):
    perm = np.concatenate([np.arange(0, HD, 2), np.arange(1, HD, 2)])
    bf = ml_dtypes.bfloat16
    xT = np.ascontiguousarray(x.reshape(S, D).T).astype(bf)
    xTt = np.concatenate(
        [_tile_rows(xT[:, c * SQT:(c + 1) * SQT]) for c in range(NSQ)],
        axis=0)
    cos2 = np.ascontiguousarray(np.vstack([cos.T, cos.T])).astype(bf)
    sin2 = np.ascontiguousarray(np.vstack([sin.T, sin.T])).astype(bf)
    # 0/1 mask for the diagonal 128-strip: allowed iff col >= partition
    binm = np.triu(np.ones((128, 128), np.float32)).astype(bf)
    swp = np.zeros((128, 128), np.float32)
    for mcol in range(64):
        swp[mcol + 64, mcol] = -1.0
    for mcol in range(64, 128):
        swp[mcol - 64, mcol] = 1.0
    swp = swp.astype(bf)
    idn = np.eye(128, dtype=np.float32).astype(bf)

    scale = 1.0 / math.sqrt(HD)
    in_maps = []
    for c in range(N_CORES):
        qcols = np.concatenate([(4 * c + hh) * HD + perm for hh in range(HL)])
        wq_c = (np.ascontiguousarray(wq[:, qcols]) * np.float32(scale)).astype(bf)
        wkv_c = np.ascontiguousarray(
            np.concatenate([wk[:, c * HD + perm], wv[:, c * HD:(c + 1) * HD]],
                           axis=1)).astype(bf)
        wo_c = wo[:, c * SQT:(c + 1) * SQT].astype(bf)
        # [D, SQT] -> [128, NK*SQT] partition-major (row g*128+p -> [p, g])
        wot_c = np.ascontiguousarray(
            wo_c.reshape(NK, 128, SQT).transpose(1, 0, 2).reshape(
                128, NK * SQT))
        in_maps.append({
            "xTt": xTt, "wqt": _tile_rows(wq_c), "wkvt": _tile_rows(wkv_c),
            "wot": wot_c,
            "cos2": cos2, "sin2": sin2, "binm": binm, "swp": swp, "idn": idn,
        })
    return in_maps


def kernel(x, wq, wk, wv, wo, cos, sin, mask, _trace=False):
    in_maps = _host_prep(np.asarray(x, np.float32), np.asarray(wq, np.float32),
                         np.asarray(wk, np.float32), np.asarray(wv, np.float32),
                         np.asarray(wo, np.float32), np.asarray(cos, np.float32),
                         np.asarray(sin, np.float32), np.asarray(mask, np.float32))
    if "nc" not in _CACHE:
        _CACHE["nc"] = build_nc()
    nc = _CACHE["nc"]
    res = run_bass_kernel_spmd(nc, in_maps, core_ids=list(range(N_CORES)),
                               trace=_trace,
                               trace_cores=list(range(N_CORES)) if _trace else None)
    out = np.empty((1, S, D), np.float32)
    for c in range(N_CORES):
        out[0, :, c * SQT:(c + 1) * SQT] = np.asarray(
            res.results[c]["out"], dtype=np.float32)
    if _trace:
        _CACHE["last_exec_time_ns"] = res.exec_time_ns
        _CACHE["last_results"] = res
    return out


# revision 15
# speedup vs baseline: 1.0802x; 1.0470x over previous
"""GQA attention (B=1, S=2048, D=4096, HQ=32, HKV=8, HD=128) + RoPE + causal,
tensor-parallel over heads on 8 TRN2 NeuronCores.

v3 strategy (vs v2): the PE sustains ~1.95 GHz and weight loads are fully
hidden, so the wins are stall/ramp elimination, not fewer LDWEIGHTS:
 - causal mask applied as a cheap bf16 0/1 triu multiply AFTER exp (one
   [128,128] pattern shared by every diagonal block) instead of a f32
   PSUM-read mask add — removes ~50us of near-saturated DVE work.
 - projection split into a kv pass (2 PSUM banks) and a q pass (4 banks):
   kv(c+1) matmuls interleave into attention(c) heads 2-3 where the PE
   otherwise stalls on the exp chain; wo(0) interleaves into attention(3).
   PSUM banks are explicitly tag-partitioned (sps2/ops2/kv2/wo1/tmp1).
 - kv(0) starts ~3us in (needs only first x + wkv chunks; wq can arrive
   ~20us later for the q pass).
All PE matmuls bf16 (f32 PSUM)."""

import math

import ml_dtypes
import numpy as np

import concourse.bass as bass
import concourse.tile as tile
from concourse import bacc, mybir
from concourse.bass_utils import run_bass_kernel_spmd

F32 = mybir.dt.float32
BF16 = mybir.dt.bfloat16

S = 2048
D = 4096
HQ, HKV, HD = 32, 8, 128
HL = HQ // 8            # 4 local q heads per core
SQT = 512               # sq tile
NSQ = S // SQT          # 4
NK = D // 128           # 32 contraction k-tiles
NSK = S // 128          # 16 sk tiles
N_CORES = 8
AF = mybir.ActivationFunctionType
ALU = mybir.AluOpType


def dedup_ldweights(nc):
    """Remove InstLdweights whose weights AP identically matches the previous
    InstLdweights on the PE queue with only InstMatmult instructions between.
    (Measured neutral on HW — loads are already hidden — but trims the PE
    queue stream.)"""
    deleted = {}
    for f in nc.m.functions:
        for bb in f.blocks:
            insts = bb.instructions
            out = []
            last_key = None
            last_name = None
            pending_edges = []
            for i in insts:
                tn = type(i).__name__
                eng = getattr(i, "engine", None)
                is_pe = eng is not None and "PE" in str(eng)
                if tn == "InstLdweights":
                    key = str(i.ins[0])
                    if key == last_key:
                        deleted[i.name] = last_name
                        pending_edges.extend(i.dependency_edges())
                        continue
                    last_key = key
                    last_name = i.name
                elif tn == "InstMatmult":
                    if getattr(i, "is_transpose", False):
                        last_key = None
                    for nm, info in pending_edges:
                        i.add_dependency(nm, info)
                    pending_edges = []
                elif is_pe:
                    last_key = None
                out.append(i)
            assert not pending_edges
            bb.instructions = out
    if deleted:
        for f in nc.m.functions:
            for bb in f.blocks:
                for i in bb.instructions:
                    try:
                        i.remap_dependency_names(deleted)
                    except Exception:
                        pass
    return len(deleted)


def build_nc():
    nc = bacc.Bacc(num_devices=N_CORES, num_swdge_queues=4)

    # host-pre-tiled inputs: every DMA reads >=4KB contiguous per partition.
    xTt = nc.declare_dram_parameter("xTt", [NSQ * 8 * 128, 4 * SQT], BF16,
                                    isOutput=False)
    wqt = nc.declare_dram_parameter("wqt", [8 * 128, 4 * HL * HD], BF16,
                                    isOutput=False)
    wkvt = nc.declare_dram_parameter("wkvt", [8 * 128, 4 * 2 * HD], BF16,
                                     isOutput=False)
    wot = nc.declare_dram_parameter("wot", [128, NK * SQT], BF16,
                                    isOutput=False)  # col shard
    cos2 = nc.declare_dram_parameter("cos2", [128, S], BF16, isOutput=False)
    sin2 = nc.declare_dram_parameter("sin2", [128, S], BF16, isOutput=False)
    binm = nc.declare_dram_parameter("binm", [128, 128], BF16, isOutput=False)
    swp = nc.declare_dram_parameter("swp", [128, 128], BF16, isOutput=False)
    idn = nc.declare_dram_parameter("idn", [128, 128], BF16, isOutput=False)
    out = nc.declare_dram_parameter("out", [S, SQT], BF16, isOutput=True)

    with tile.TileContext(nc) as tc:
        with tc.tile_pool(name="const", bufs=1) as constp, \
             tc.tile_pool(name="wqp", bufs=3) as wqp, \
             tc.tile_pool(name="wkvp", bufs=1) as wkvp, \
             tc.tile_pool(name="wob", bufs=1) as wob, \
             tc.tile_pool(name="xp", bufs=8) as xp, \
             tc.tile_pool(name="qp", bufs=2) as qp, \
             tc.tile_pool(name="ep", bufs=6) as ep, \
             tc.tile_pool(name="eap", bufs=2) as eap, \
             tc.tile_pool(name="tp", bufs=2) as tp, \
             tc.tile_pool(name="agp", bufs=6) as agp, \
             tc.tile_pool(name="op", bufs=2) as opool, \
             tc.tile_pool(name="outp", bufs=2) as outp, \
             tc.tile_pool(name="ps", bufs=1, space="PSUM") as ps, \
             tc.tile_pool(name="dram", bufs=1, space="DRAM") as dramp:

            # PSUM bank partitioning (8 banks total), all [128,512] f32:
            #   sps(2): score tiles   ops(2): o_ps per head / q0,q1 accs
            #   kv (2): k,v accs / q2,q3 accs   wo(1): wo accumulator
            #   tmp(1): rope-swap temp, v-transpose temp, softmax-sum temp
            def pt(tag, name):
                bufs = {"sps": 2, "ops": 2, "kv": 2, "wo": 1, "tmp": 1}[tag]
                return ps.tile([128, SQT], F32, tag=tag, bufs=bufs, name=name)

            # ---- initial DMAs.  The kv(0) pass consumes x+wkv chunks at
            # ---- ~235 GB/s, so the first tile's feed is spread across all
            # ---- three queues interleaved in consumption order.
            wkv_t = [wkvp.tile([128, 4 * 2 * HD], BF16, tag=f"wkv{j}",
                               name=f"wkv{j}") for j in range(8)]
            xts = {}
            for j in range(8):
                xts[(0, j)] = xp.tile([128, 4 * SQT], BF16, tag="xt",
                                      name=f"x0{j}")

            def x0dma(j, eng):
                eng.dma_start(out=xts[(0, j)],
                              in_=xTt[j * 128:(j + 1) * 128, :])

            def wkvdma(j, eng):
                eng.dma_start(out=wkv_t[j], in_=wkvt[j * 128:(j + 1) * 128, :])

            for j, eng in ((0, nc.gpsimd), (2, nc.gpsimd), (4, nc.gpsimd),
                           (6, nc.gpsimd)):
                x0dma(j, eng)
            wkvdma(0, nc.scalar)
            wkvdma(1, nc.sync)
            x0dma(1, nc.sync)
            wkvdma(2, nc.scalar)
            wkvdma(3, nc.sync)
            x0dma(3, nc.scalar)
            wkvdma(4, nc.scalar)
            wkvdma(5, nc.sync)
            x0dma(5, nc.sync)
            wkvdma(6, nc.scalar)
            wkvdma(7, nc.sync)
            x0dma(7, nc.scalar)

            def wq_stream(c):
                # even chunks ride gpsimd (fast, behind x), odd chunks the
                # scalar hw queue so the stream lands in ~half the time
                ts = [wqp.tile([128, 4 * HL * HD], BF16, tag="wq",
                               name=f"wqs{c}{j}") for j in range(8)]
                for j in range(8):
                    eng = nc.gpsimd if j % 2 == 0 else nc.scalar
                    eng.dma_start(out=ts[j],
                                  in_=wqt[j * 128:(j + 1) * 128, :])
                return ts

            wqs = {0: wq_stream(0)}

            # constants ride gpsimd behind the x evens: rope(k0) needs
            # cos/sin/swp at ~25us, before the scalar/sync queues drain
            cos_sb = constp.tile([128, S], BF16)
            nc.gpsimd.dma_start(out=cos_sb, in_=cos2[:, :])
            sin_sb = constp.tile([128, S], BF16)
            nc.gpsimd.dma_start(out=sin_sb, in_=sin2[:, :])
            swp_sb = constp.tile([128, 128], BF16)
            nc.gpsimd.dma_start(out=swp_sb, in_=swp[:, :])
            bin_sb = constp.tile([128, 128], BF16)
            nc.gpsimd.dma_start(out=bin_sb, in_=binm[:, :])
            idn_sb = constp.tile([128, 128], BF16)
            nc.gpsimd.dma_start(out=idn_sb, in_=idn[:, :])
            ones_f = constp.tile([128, 128], F32)
            nc.vector.memset(ones_f, 1.0)
            allones = constp.tile([128, 128], BF16)
            nc.scalar.activation(allones, ones_f, AF.Copy)

            kT_sb = constp.tile([128, S], BF16)      # kv head, feature-major
            v_sb = constp.tile([128, S], BF16)       # [sk%128, (sk//128)*128+d]

            wo_sb = wob.tile([128, NK * SQT], BF16)

            ag_in = [dramp.tile([HL * 128, SQT], BF16, name=f"agin{c}")
                     for c in range(NSQ)]
            ag_out = [dramp.tile([HQ * 128, SQT], BF16, addr_space="Shared",
                                 name=f"agout{c}") for c in range(NSQ)]

            def rope(ps_t, cos_cols, sin_cols, dst):
                """dst[bf16 128xSQT] = cos*ps + signed-half-swap(sin*ps)."""
                t2 = tp.tile([128, SQT], BF16, tag="t2")
                nc.vector.tensor_tensor(out=t2, in0=ps_t, in1=sin_cols,
                                        op=ALU.mult)
                t2s = pt("tmp", "t2s")
                nc.tensor.matmul(t2s, swp_sb, t2, start=True, stop=True)
                t1 = tp.tile([128, SQT], BF16, tag="t1")
                nc.vector.tensor_tensor(out=t1, in0=ps_t, in1=cos_cols,
                                        op=ALU.mult)
                nc.vector.tensor_tensor(out=dst, in0=t1, in1=t2s, op=ALU.add)

            # ---- filler streams: emit_filler(n) emits up to n deferred PE
            # ---- matmuls (kv proj of the next tile, or wo(0) during attn(3))
            filler = []

            def emit_filler(n):
                for _ in range(n):
                    if not filler:
                        return
                    filler.pop(0)()

            def kv_proj_ops(c):
                """Return list of closures: kv projection matmuls for tile c
                (2 PSUM banks) + the trailing v drain."""
                k_ps = pt("kv", f"kps{c}")
                v_ps = pt("kv", f"vps{c}")
                ops = []
                for kt in range(NK):
                    t = kt % 4

                    def mm(kt=kt, t=t):
                        xt = xts[(c, kt // 4)][:, t * SQT:(t + 1) * SQT]
                        nc.tensor.matmul(
                            k_ps,
                            wkv_t[kt // 4][:, t * 2 * HD:t * 2 * HD + HD],
                            xt, start=(kt == 0), stop=(kt == NK - 1))
                        nc.tensor.matmul(
                            v_ps,
                            wkv_t[kt // 4][:, t * 2 * HD + HD:(t + 1) * 2 * HD],
                            xt, start=(kt == 0), stop=(kt == NK - 1))
                    ops.append(mm)
                return ops, k_ps, v_ps

            def q_phase(c, k_ps, v_ps):
                """rope k, transpose v, project+rope q0/q1 (q2/q3 roped lazily
                during attention).  Returns (qT_sb, q_ps list)."""
                s0 = c * SQT
                cse = (slice(None), slice(s0, s0 + SQT))
                vt_sb = tp.tile([128, SQT], BF16, tag="vt", bufs=2)
                nc.scalar.activation(vt_sb, v_ps, AF.Copy)
                rope(k_ps, cos_sb[cse], sin_sb[cse], kT_sb[cse])
                for sb in range(SQT // 128):
                    vp = ps.tile([128, 128], BF16, tag="tmp", bufs=1,
                                 name="vtp")
                    nc.tensor.transpose(vp, vt_sb[:, sb * 128:(sb + 1) * 128],
                                        idn_sb)
                    nc.scalar.activation(
                        v_sb[:, (4 * c + sb) * 128:(4 * c + sb + 1) * 128],
                        vp, AF.Copy)
                q_ps = [pt("ops", f"qps{c}0"), pt("ops", f"qps{c}1"),
                        pt("kv", f"qps{c}2"), pt("kv", f"qps{c}3")]
                for kt in range(NK):
                    t = kt % 4
                    xt = xts[(c, kt // 4)][:, t * SQT:(t + 1) * SQT]
                    for h in range(HL):
                        nc.tensor.matmul(
                            q_ps[h],
                            wqs[c][kt // 4][:, t * HL * HD + h * 128:
                                            t * HL * HD + (h + 1) * 128],
                            xt, start=(kt == 0), stop=(kt == NK - 1))
                qT_sb = qp.tile([128, HL, SQT], BF16, tag="qT")
                rope(q_ps[0], cos_sb[cse], sin_sb[cse], qT_sb[:, 0, :])
                rope(q_ps[1], cos_sb[cse], sin_sb[cse], qT_sb[:, 1, :])
                return qT_sb, q_ps

            def attention_head(c, h, qT_sb, fin_prev):
                """One head's scores/exp/PV; returns a finalize closure the
                caller emits later.  Diagonal blocks: exp on raw scores then a
                bf16 0/1 triu multiply (same [128,128] pattern every block).
                e accumulated f32, split DVE (even blocks) / GpSimd (odd)."""
                nsk_here = 4 * c + 4
                o_ps = pt("ops", f"o{c}{h}")
                e_acc_d = eap.tile([128, SQT], F32, tag="eaccd")
                e_acc_g = eap.tile([128, SQT], F32, tag="eaccg")
                if c == 0:
                    nc.gpsimd.memset(e_acc_g, 0.0)

                def flush(pending):
                    pe, pc, pk = pending
                    nc.tensor.matmul(
                        o_ps[:, pc:], v_sb[:, pk * 128:(pk + 1) * 128],
                        pe[:, pc:], start=(pk == 0),
                        stop=(pk == nsk_here - 1))

                pending = []  # (e_sb, col0, kt2) awaiting PV
                for kt2 in range(nsk_here):
                    m = kt2 - 4 * c
                    col0 = 128 * m if m > 0 else 0
                    s_ps = pt("sps", "s_ps")
                    nc.tensor.matmul(
                        s_ps[:, col0:], kT_sb[:, kt2 * 128:(kt2 + 1) * 128],
                        qT_sb[:, h, col0:], start=True, stop=True)
                    e_sb = ep.tile([128, SQT], BF16, tag="e")
                    nc.scalar.activation(e_sb[:, col0:], s_ps[:, col0:],
                                         AF.Exp)
                    if m >= 0:
                        # zero the strictly-upper triangle of the diagonal
                        # 128-col strip (bf16 mult, ~4x cheaper than the f32
                        # PSUM mask add it replaces)
                        nc.vector.tensor_tensor(
                            out=e_sb[:, col0:col0 + 128],
                            in0=e_sb[:, col0:col0 + 128],
                            in1=bin_sb, op=ALU.mult)
                    eng = nc.vector if kt2 % 2 == 0 else nc.gpsimd
                    acc = e_acc_d if kt2 % 2 == 0 else e_acc_g
                    if kt2 == 0 or (kt2 == 1 and c > 0):
                        eng.tensor_copy(out=acc, in_=e_sb)
                    else:
                        eng.tensor_tensor(out=acc[:, col0:],
                                          in0=acc[:, col0:],
                                          in1=e_sb[:, col0:], op=ALU.add)
                    pending.append((e_sb, col0, kt2))
                    if len(pending) > 3:
                        flush(pending.pop(0))
                    if kt2 == 2 and fin_prev is not None:
                        fin_prev()
                    # kv fillers touch the kv PSUM slots, free only once
                    # q2/q3 are roped (ends of heads 0/1); wo fillers (last
                    # tile) have no such hazard and can start at head 0
                    if h >= 2 or c == NSQ - 1:
                        emit_filler(2 if c < 2 else 4)
                for p in pending:
                    flush(p)

                def finalize():
                    ea_bf = eap.tile([128, SQT], BF16, tag="eaccb")
                    nc.vector.tensor_tensor(out=ea_bf, in0=e_acc_d,
                                            in1=e_acc_g, op=ALU.add)
                    sum_ps = pt("tmp", f"sb{c}{h}")
                    nc.tensor.matmul(sum_ps, allones, ea_bf,
                                     start=True, stop=True)
                    rec_sb = opool.tile([128, SQT], F32, tag="rcb")
                    nc.vector.reciprocal_approx_fast(out=rec_sb, in_=sum_ps)
                    on_sb = opool.tile([128, SQT], BF16, tag="on")
                    nc.vector.tensor_tensor(out=on_sb, in0=rec_sb, in1=o_ps,
                                            op=ALU.mult)
                    nc.gpsimd.dma_start(
                        out=ag_in[c][h * 128:(h + 1) * 128, :], in_=on_sb)
                return finalize

            def wo_ops(c, tag, wait_ms):
                """Closures for wo(c): 4 row-blocks x 32 contraction matmuls.
                tile_wait_until pins the fetches AND matmuls at/after the sim
                time their AllGather is genuinely done — the scheduler's sim
                treats collectives as instant and otherwise hoists these into
                earlier phases, where on real HW they block the in-order PE
                queue on AG completion (~30us skew) [seen: 32us stall]."""
                # fetch as [128 feat, 8 g-blocks, 512 sq] chunks: 1KB
                # descriptors (vs 256B column-sliced), split over all three
                # DMA queues so a 4MB phase fetch lands in ~13us
                ag_ts = []
                for gq in range(4):
                    ag_t = agp.tile([128, 8, SQT], BF16, tag="ag",
                                    name=f"agt{c}{gq}")
                    eng = (nc.sync, nc.scalar, nc.gpsimd)[gq % 3]
                    with tc.tile_wait_until(wait_ms):
                        eng.dma_start(
                            out=ag_t,
                            in_=ag_out[c][gq * 1024:(gq + 1) * 1024, :]
                            .rearrange("(g p) n -> p g n", p=128))
                    ag_ts.append(ag_t)
                ops = []
                state = {}
                for mt in range(4):
                    for g in range(NK):
                        def mm(mt=mt, g=g):
                            with tc.tile_wait_until(wait_ms + 0.01):
                                if g == 0:
                                    state["o1"] = pt(tag, f"wops{c}{mt}")
                                nc.tensor.matmul(
                                    state["o1"],
                                    ag_ts[g // 8][:, g % 8,
                                                  mt * 128:(mt + 1) * 128],
                                    wo_sb[:, g * SQT:(g + 1) * SQT],
                                    start=(g == 0), stop=(g == NK - 1))
                                if g == NK - 1:
                                    ob = outp.tile([128, SQT], BF16, tag="ob")
                                    nc.scalar.activation(ob, state["o1"],
                                                         AF.Copy)
                                    nc.scalar.dma_start(
                                        out=out[c * SQT + mt * 128:
                                                c * SQT + (mt + 1) * 128, :],
                                        in_=ob)
                        ops.append(mm)
                return ops

            # ================= main schedule =================
            kv_ops, k_ps, v_ps = kv_proj_ops(0)
            for op_ in kv_ops:
                op_()

            for c in range(NSQ):
                s0 = c * SQT
                cse = (slice(None), slice(s0, s0 + SQT))
                qT_sb, q_ps = q_phase(c, k_ps, v_ps)

                # prefetch next tile's x (consumed by interleaved kv proj)
                # and wq (needed at q_phase(c+1))
                if c + 1 < NSQ:
                    for j in range(8):
                        xts[(c + 1, j)] = xp.tile([128, 4 * SQT], BF16,
                                                  tag="xt", name=f"x{c+1}{j}")
                        xeng = nc.gpsimd if j % 2 == 0 else nc.sync
                        xeng.dma_start(
                            out=xts[(c + 1, j)],
                            in_=xTt[((c + 1) * 8 + j) * 128:
                                    ((c + 1) * 8 + j + 1) * 128, :])
                    wqs[c + 1] = wq_stream(c + 1)
                    kv_ops, k_ps, v_ps = kv_proj_ops(c + 1)
                    filler.extend(kv_ops)
                if c == 1:
                    nc.gpsimd.dma_start(out=wo_sb, in_=wot[:, :])
                if c == NSQ - 1:
                    filler.extend(wo_ops(0, "wo", 0.19))

                fin = None
                for h in range(HL):
                    fin = attention_head(c, h, qT_sb, fin)
                    if h + 2 < HL:
                        rope(q_ps[h + 2], cos_sb[cse], sin_sb[cse],
                             qT_sb[:, h + 2, :])
                fin()
                emit_filler(len(filler))  # drain leftover before AG

                nc.gpsimd.collective_compute(
                    "AllGather", ALU.bypass,
                    replica_groups=[list(range(N_CORES))],
                    ins=[ag_in[c].opt()], outs=[ag_out[c].opt()])

            # tail: remaining wo phases (wo(0) ran inside attention(3)).
            # Accumulators ride the 'kv' ring (idle after attention(3)) so
            # consecutive row-blocks pipeline 2-deep across the ACT drain.
            for cw, wms in ((1, 0.26), (2, 0.29), (3, 0.32)):
                for op_ in wo_ops(cw, "kv", wms):
                    op_()

    dedup_ldweights(nc)
    nc.finalize()
    return nc


_CACHE = {}


def _tile_rows(a, chunk=4):
    """[D, C] -> [D//(128*chunk) * 128, chunk*C]: row blocks partition-major
    so each DMA partition reads chunk*C contiguous elements."""
    dd, cc = a.shape
    nj = dd // (128 * chunk)
    return np.ascontiguousarray(
        a.reshape(nj, chunk, 128, cc).transpose(0, 2, 1, 3).reshape(
            nj * 128, chunk * cc))


def _host_prep(x, wq, wk, wv, wo, cos, sin, mask):
    perm = np.concatenate([np.arange(0, HD, 2), np.arange(1, HD, 2)])
    bf = ml_dtypes.bfloat16
    xT = np.ascontiguousarray(x.reshape(S, D).T).astype(bf)
    xTt = np.concatenate(
        [_tile_rows(xT[:, c * SQT:(c + 1) * SQT]) for c in range(NSQ)],
        axis=0)
    cos2 = np.ascontiguousarray(np.vstack([cos.T, cos.T])).astype(bf)
    sin2 = np.ascontiguousarray(np.vstack([sin.T, sin.T])).astype(bf)
    # 0/1 mask for the diagonal 128-strip: allowed iff col >= partition
    binm = np.triu(np.ones((128, 128), np.float32)).astype(bf)
    swp = np.zeros((128, 128), np.float32)
    for mcol in range(64):
        swp[mcol + 64, mcol] = -1.0
    for mcol in range(64, 128):
        swp[mcol - 64, mcol] = 1.0
    swp = swp.astype(bf)
    idn = np.eye(128, dtype=np.float32).astype(bf)

    scale = 1.0 / math.sqrt(HD)
    in_maps = []
    for c in range(N_CORES):
        qcols = np.concatenate([(4 * c + hh) * HD + perm for hh in range(HL)])
        wq_c = (np.ascontiguousarray(wq[:, qcols]) * np.float32(scale)).astype(bf)
        wkv_c = np.ascontiguousarray(
            np.concatenate([wk[:, c * HD + perm], wv[:, c * HD:(c + 1) * HD]],
                           axis=1)).astype(bf)
        wo_c = wo[:, c * SQT:(c + 1) * SQT].astype(bf)
        # [D, SQT] -> [128, NK*SQT] partition-major (row g*128+p -> [p, g])
        wot_c = np.ascontiguousarray(
            wo_c.reshape(NK, 128, SQT).transpose(1, 0, 2).reshape(
                128, NK * SQT))
        in_maps.append({
            "xTt": xTt, "wqt": _tile_rows(wq_c), "wkvt": _tile_rows(wkv_c),
            "wot": wot_c,
            "cos2": cos2, "sin2": sin2, "binm": binm, "swp": swp, "idn": idn,
        })
    return in_maps


def kernel(x, wq, wk, wv, wo, cos, sin, mask, _trace=False):
    in_maps = _host_prep(np.asarray(x, np.float32), np.asarray(wq, np.float32),
                         np.asarray(wk, np.float32), np.asarray(wv, np.float32),
                         np.asarray(wo, np.float32), np.asarray(cos, np.float32),
                         np.asarray(sin, np.float32), np.asarray(mask, np.float32))
    if "nc" not in _CACHE:
        _CACHE["nc"] = build_nc()
    nc = _CACHE["nc"]
    res = run_bass_kernel_spmd(nc, in_maps, core_ids=list(range(N_CORES)),
                               trace=_trace,
                               trace_cores=list(range(N_CORES)) if _trace else None)
    out = np.empty((1, S, D), np.float32)
    for c in range(N_CORES):
        out[0, :, c * SQT:(c + 1) * SQT] = np.asarray(
            res.results[c]["out"], dtype=np.float32)
    if _trace:
        _CACHE["last_exec_time_ns"] = res.exec_time_ns
        _CACHE["last_results"] = res
    return out
